# revision 92
# baseline (speedup 1.0000x reference)
"""Local sliding-window attention block (MQA + partial RoPE) on 8 TRN2 cores.

Sharding: 2 batches x 4 sequence chunks of 512 queries each. Each core
computes q/k/v projections for its chunk (keys include a 512-token halo),
windowed attention (window=512, causal), and the o-projection for its own
query rows — so the host-side unshard is a pure concatenation.

On-chip layout: everything transposed (feature dim on partitions).
  xT[d, pos]  ->  Q^T[dh, q] / K^T[dh, k] (RoPE'd)  ->  S^T[k, q]
  -> exp -> P^T[k, q] (bf16, multiplicative 0/1 masks)
  -> O^T[dv, q] = V.T-matmul  -> normalized by softmax denominators
     (partition_all_reduce on GPSIMD)  -> used directly as lhsT of o-proj.
All matmuls bf16 inputs, fp32 PSUM accumulation.

Schedule notes (tuned against the CoreSim cost model):
 - weight/const DMAs are host-pre-laid-out to match SBUF (>=512B rows,
   full DMA bus rate) and ordered so the K/V projection starts on the
   first xT tile; per-head Wq streams 2 heads ahead with 3 buffers.
 - V is projected transposed (one PSUM accumulation group per bank) and
   flipped to [pos, dv] tiles with DMA xbar transposes.
 - per head the PE stream is software-pipelined: scores(t0,t1) -> next
   head's q-projection -> scores(t2,t3) -> all AV matmuls, so the
   exp/mask chain on Act/DVE hides under the q-projection. The last
   head interleaves heads 0..14 of the first two o-proj units instead.
 - softmax sig-reduction is an fp16 add-tree (2-byte packed DVE ops run
   2x, first add on GPSIMD) instead of a strided TensorReduce.
 - rope half-swaps ride the GPSIMD SWDGE queue so the SP load queue is
   never blocked behind data-dependent transfers.
 - the whole o-projection runs out of the score-PSUM banks (no pool
   transition barrier); the final unit is split so the trailing
   bias-add + store latency shrinks.
"""

import numpy as np
import ml_dtypes

BF16 = ml_dtypes.bfloat16
F8 = ml_dtypes.float8_e4m3

B, L, D = 2, 2048, 2048
H, HD = 16, 128
ROPE_DIMS, HALF = 64, 32
WINDOW = 512
ROPE_BASE = 10000.0
SCALE = HD ** -0.5
# fp8 quantization scales: x is quantized at 16x, weights at 2048x, so every
# projection PSUM carries a 2^15 factor that the exp-scale / host descale absorb
SX = 16.0
SW = 2048.0
S2 = SX * SW  # 2^15
# attention outputs are quantized fp8 at 64x (the V psum->sbuf copy descales by
# SO/S2 so the AV output lands at 64*true already); Wo fp8 at 2048x
SO = 64.0
SWO = 2048.0

CHUNK = 512            # queries per core
NK = 1024              # keys (incl. halo) per core
NQT = CHUNK // 128     # 4 local query tiles
NKT = NK // 128        # 8 local key tiles
NSIG = 5               # key tiles in window per query tile
NDT = D // 128         # 16 contraction tiles over embedding dim
NPAIR = NDT // 2       # 8 dt pairs (256-deep DoubleRow contraction units)
DN = D // 512          # 4 o-proj column blocks

_PROGRAM = None


def _build_program():
    from contextlib import ExitStack
    import concourse.bass as bass
    import concourse.mybir as mybir
    import concourse.tile as tile
    import concourse.bass_isa as bass_isa
    from concourse import bacc

    fp32 = mybir.dt.float32
    fp16 = mybir.dt.float16
    bf16 = mybir.dt.bfloat16
    f8e4 = mybir.dt.float8e4
    DR = mybir.MatmulPerfMode.DoubleRow
    AF = mybir.ActivationFunctionType

    nc = bacc.Bacc(None, target_bir_lowering=False)

    # fp8 operands carry (hi, lo) compensation pairs: x tiles are laid out
    # [p, dt, (hi, lo), pos], weight tiles [p, dt, (lo, hi), col] so that a
    # single DoubleRow matmul over the hl axis yields the Wl.T@Xh + Wh.T@Xl
    # cross terms, and a DoubleRow over a dt pair at hl=hi yields the main
    # term with a 256-deep contraction at half the per-row cost.
    xT_d = nc.dram_tensor("xT8", [128, NDT, 2, NK], f8e4, kind="ExternalInput")
    wq_d = nc.dram_tensor("Wq", [H, 128, NDT, 2, 128], f8e4, kind="ExternalInput")
    wk_d = nc.dram_tensor("Wk", [128, NDT, 2, HD], f8e4, kind="ExternalInput")
    wv_d = nc.dram_tensor("Wv", [128, NDT, 2, HD], f8e4, kind="ExternalInput")
    wo_d = nc.dram_tensor("Wo", [DN, 128, H, 2, 512], f8e4, kind="ExternalInput")
    bo_d = nc.dram_tensor("bo", [1, D], bf16, kind="ExternalInput")
    cos_d = nc.dram_tensor("cosT", [ROPE_DIMS, NK], bf16, kind="ExternalInput")
    sin_d = nc.dram_tensor("sinT", [ROPE_DIMS, NK], bf16, kind="ExternalInput")
    # boundary-tile masks only: [k, t, q] = sig0 strict-upper triangle and
    # [k, NQT, q] = sig4 causal lower triangle. Interior sig tiles are
    # all-ones except fully-padded ones (exp(0)=1 everywhere). On cores whose
    # sig0 tile is padding, es0 is exactly 1, so the sig0 "mask" doubles as
    # the denominator correction: a constant -npad(t) tile whose masked
    # product cancels the padded interior tiles' inflation in the add tree
    # (its AV contribution is zero because V is zero on padded keys).
    msk_d = nc.dram_tensor("masks", [128, NQT + 1, 128], bf16, kind="ExternalInput")
    out_d = nc.dram_tensor("out", [CHUNK, D], bf16, kind="ExternalOutput")

    def _rope(pool, out_bf, ps, cos2, sin2m, eng=None):
        """out[0:64] = rotary(ps[0:64]); out[64:128] = ps[64:128].

        ps fp32 PSUM, out bf16. cos2/sin2m bf16 [64, n] row tables
        (rows [0:32]==[32:64]==cos; sin rows [0:32]=-sin, [32:64]=+sin).
        The half-swap goes through two partition-shifting DMAs; the
        PSUM->SBUF casts run on Act, the bf16 elementwise math runs on
        DVE in the 2x packed-16-bit mode.
        """
        n = cos2.shape[-1]
        sb64 = pool.tile([ROPE_DIMS, n], bf16, tag="rope_sb64")
        nc.scalar.copy(sb64, ps[0:ROPE_DIMS])
        ss = pool.tile([ROPE_DIMS, n], bf16, tag="rope_ss")
        # SWDGE (gpsimd) queue: keeps these data-dependent shuffles out of
        # the SP load queue so weight streaming is never blocked behind them
        nc.gpsimd.dma_start(out=ss[0:HALF], in_=sb64[HALF:ROPE_DIMS])
        nc.gpsimd.dma_start(out=ss[HALF:ROPE_DIMS], in_=sb64[0:HALF])
        t1 = pool.tile([ROPE_DIMS, n], bf16, tag="rope_t1")
        eng = eng or nc.vector
        eng.tensor_mul(t1, sb64, cos2)
        eng.tensor_mul(ss, ss, sin2m)
        eng.tensor_add(out_bf[0:ROPE_DIMS], t1, ss)
        nc.scalar.copy(out_bf[ROPE_DIMS:HD], ps[ROPE_DIMS:HD])

    with tile.TileContext(nc) as tc, ExitStack() as ctx:
        p_const = ctx.enter_context(tc.tile_pool(name="const", bufs=1))
        p_xt = ctx.enter_context(tc.tile_pool(name="xt", bufs=1))
        p_kv = ctx.enter_context(tc.tile_pool(name="kv", bufs=1))
        p_wq = ctx.enter_context(tc.tile_pool(name="wq", bufs=3))
        p_qt = ctx.enter_context(tc.tile_pool(name="qt", bufs=3))
        p_es = ctx.enter_context(tc.tile_pool(name="es", bufs=6))
        p_red = ctx.enter_context(tc.tile_pool(name="red", bufs=8))
        p_dn = ctx.enter_context(tc.tile_pool(name="dn", bufs=2))
        p_tmp = ctx.enter_context(tc.tile_pool(name="tmp", bufs=4))
        p_otn = ctx.enter_context(tc.tile_pool(name="otn", bufs=1))
        p_wo = ctx.enter_context(tc.tile_pool(name="wo", bufs=3))
        p_ob = ctx.enter_context(tc.tile_pool(name="ob", bufs=4))

        # ---- loads: all transfers serialize on the DMA engines, so the order
        # below IS the startup schedule; kv/q0 matmuls are paced pair-by-pair
        wk_sb = p_const.tile([128, NDT, 2, HD], f8e4, tag="wk")
        nc.sync.dma_start(out=wk_sb, in_=wk_d[:])

        # x in 8 dt-pair tiles [p, 2(dt), 2(hl), pos] so the first DoubleRow
        # starts after the first pair lands
        xp = []
        for i in range(NPAIR):
            t_ = p_xt.tile([128, 2, 2, NK], f8e4, tag=f"xp{i}")
            xp.append(t_)
        nc.sync.dma_start(out=xp[0], in_=xT_d[:, 0:2])

        wv_sb = p_const.tile([128, NDT, 2, HD], f8e4, tag="wv")
        nc.sync.dma_start(out=wv_sb, in_=wv_d[:])

        wq_sb = []
        for h in range(H):
            t_ = p_wq.tile([128, NDT, 2, 128], f8e4, tag="wq", name=f"wq{h}")
            wq_sb.append(t_)
        nc.sync.dma_start(out=wq_sb[0], in_=wq_d[0])

        cos_sb = p_const.tile([ROPE_DIMS, NK], bf16, tag="cos")
        sin_sb = p_const.tile([ROPE_DIMS, NK], bf16, tag="sin")
        msk_sb = p_const.tile([128, NQT + 1, 128], bf16, tag="msk")

        for i in range(1, NPAIR):
            nc.sync.dma_start(out=xp[i], in_=xT_d[:, 2 * i:2 * i + 2])
            if i == 3:
                nc.sync.dma_start(out=cos_sb, in_=cos_d[:])
                nc.sync.dma_start(out=sin_sb, in_=sin_d[:])
            elif i == 5:
                nc.sync.dma_start(out=wq_sb[1], in_=wq_d[1])
                nc.sync.dma_start(out=msk_sb, in_=msk_d[:])
        nc.sync.dma_start(out=wq_sb[2], in_=wq_d[2])

        # ---- K^T (RoPE'd) and V projections, dt-outer so PE starts on the
        # first xT tile while the rest stream in; head-0 q-projection is
        # folded in before the PSUM pool swap so PE never drains ----
        kt = p_kv.tile([128, NK], bf16, tag="kt")
        vt_sb = p_kv.tile([128, NK], bf16, tag="vt")
        v_sb = []
        for s in range(NKT):
            t_ = p_kv.tile([128, HD], bf16, tag=f"v{s}")
            v_sb.append(t_)
        qt_sb = []
        for h in range(H):
            t_ = p_qt.tile([128, CHUNK], bf16, tag="qt", name=f"qt{h}")
            qt_sb.append(t_)

        def q_main(psq, h, p_, start):
            nc.tensor.matmul(
                psq, wq_sb[h][:, 2 * p_:2 * p_ + 2, 1, :],
                xp[p_][:, :, 0, CHUNK:NK],
                start=start, stop=False, perf_mode=DR,
            )

        def q_corr(psq, h, dt, stop):
            p_, s_ = divmod(dt, 2)
            nc.tensor.matmul(
                psq, wq_sb[h][:, dt, :, :], xp[p_][:, s_, :, CHUNK:NK],
                start=False, stop=stop, perf_mode=DR,
            )

        def q_rope(psq, h):
            # rope stays on DVE: its output gates head h's scores two heads
            # later, and the Pool FIFO is backed up behind reduce/quantize work
            _rope(p_tmp, qt_sb[h], psq, cos_sb[:, CHUNK:NK], sin_sb[:, CHUNK:NK])

        def q_proj(ps_qp, h):
            psq = ps_qp.tile([128, CHUNK], fp32, tag="ps_q")
            for p_ in range(NPAIR):
                q_main(psq, h, p_, p_ == 0)
            for dt in range(NDT):
                q_corr(psq, h, dt, dt == NDT - 1)
            q_rope(psq, h)

        with tc.tile_pool(name="ps_q", bufs=2, space=bass.MemorySpace.PSUM) as ps_qp:
            with tc.tile_pool(
                name="ps_kv", bufs=1, space=bass.MemorySpace.PSUM
            ) as ps_kv:
                ps_k = [
                    ps_kv.tile([128, 512], fp32, tag=f"ps_k{i}", name=f"ps_k{i}")
                    for i in range(2)
                ]
                # V is computed transposed (one PSUM accumulation group per
                # bank — concurrent groups within a bank are illegal) and
                # tile-transposed to [pos, dv] via the DMA xbar afterwards
                ps_vt = [
                    ps_kv.tile([128, 512], fp32, tag=f"ps_vt{i}", name=f"ps_vt{i}")
                    for i in range(2)
                ]
                # kv + head-0 q matmuls interleaved pair-by-pair, paced by the
                # x DMA stream (13 DoubleRows per pair ~= one chunk DMA)
                psq0 = ps_qp.tile([128, CHUNK], fp32, tag="ps_q")
                for p_ in range(NPAIR):
                    st = p_ == 0
                    for nh in range(2):
                        cols = slice(nh * 512, (nh + 1) * 512)
                        nc.tensor.matmul(
                            ps_k[nh], wk_sb[:, 2 * p_:2 * p_ + 2, 1, :],
                            xp[p_][:, :, 0, cols], start=st, stop=False,
                            perf_mode=DR,
                        )
                        nc.tensor.matmul(
                            ps_vt[nh], wv_sb[:, 2 * p_:2 * p_ + 2, 1, :],
                            xp[p_][:, :, 0, cols], start=st, stop=False,
                            perf_mode=DR,
                        )
                    # kv corrs before the q0 work: they only need wk/wv + x,
                    # so the first pairs don't stall on the wq0 stream
                    for s_ in range(2):
                        dt = 2 * p_ + s_
                        sp = dt == NDT - 1
                        for nh in range(2):
                            cols = slice(nh * 512, (nh + 1) * 512)
                            nc.tensor.matmul(
                                ps_k[nh], wk_sb[:, dt, :, :],
                                xp[p_][:, s_, :, cols], start=False, stop=sp,
                                perf_mode=DR,
                            )
                            nc.tensor.matmul(
                                ps_vt[nh], wv_sb[:, dt, :, :],
                                xp[p_][:, s_, :, cols], start=False, stop=sp,
                                perf_mode=DR,
                            )
                    q_main(psq0, 0, p_, st)
                    for s_ in range(2):
                        q_corr(psq0, 0, 2 * p_ + s_, 2 * p_ + s_ == NDT - 1)
                q_rope(psq0, 0)
                for nh in range(2):
                    cols = slice(nh * 512, (nh + 1) * 512)
                    _rope(p_tmp, kt[:, cols], ps_k[nh],
                          cos_sb[:, cols], sin_sb[:, cols])
                    # scaled copy on Act: divides out the 2^15 fp8 psum factor
                    # and bakes in the SO=64 scale the otn fp8 quantization
                    # wants, so the attention-output path needs no extra ops
                    nc.scalar.activation(
                        vt_sb[:, cols], ps_vt[nh], AF.Copy, scale=SO / S2
                    )
                for c in range(NKT):
                    nc.sync.dma_start_transpose(
                        out=v_sb[c], in_=vt_sb[:, c * 128:(c + 1) * 128]
                    )
                # head-1 q-projection keeps PE busy while K-rope / V copies
                # drain the kv PSUM tiles (head-0 is folded into the kv loop)
                q_proj(ps_qp, 1)

            # ---- per-head attention (software-pipelined PE stream) ----
            # attention outputs in fp8 (hi, lo) head-pair tiles, ready to be
            # DoubleRow o-proj operands: [dv, 2(head-in-pair), 2(hi/lo), q]
            otn_p = []
            for j in range(H // 2):
                t_ = p_otn.tile([128, 2, 2, CHUNK], f8e4, tag=f"otnp{j}")
                otn_p.append(t_)

            with (
                tc.tile_pool(name="ps_s", bufs=2, space=bass.MemorySpace.PSUM) as ps_sp,
                tc.tile_pool(name="ps_o", bufs=2, space=bass.MemorySpace.PSUM) as ps_op,
            ):
                bias_sb = p_const.tile([128, D], bf16, tag="bias")
                wo_tiles = []
                for n in range(DN):
                    t_ = p_wo.tile([128, H, 2, 512], f8e4, tag="wo", name=f"wo{n}")
                    wo_tiles.append(t_)

                def oproj_mains(pso, n, tq, hs, js, start):
                    for j in js:
                        nc.tensor.matmul(
                            pso, otn_p[j][:, :, 0, tq],
                            wo_tiles[n][:, 2 * j:2 * j + 2, 1, hs],
                            start=(start and j == js[0]), stop=False,
                            perf_mode=DR,
                        )

                def oproj_corrs(pso, n, tq, hs, h2s, stop):
                    for h2 in h2s:
                        j, s_ = divmod(h2, 2)
                        nc.tensor.matmul(
                            pso, otn_p[j][:, s_, :, tq],
                            wo_tiles[n][:, h2, :, hs],
                            start=False, stop=(stop and h2 == h2s[-1]),
                            perf_mode=DR,
                        )

                pso_br = []
                u_tail = []

                def oproj_partial(i, part):
                    # PE executes in queue order, so each part may only
                    # touch otn written by strictly earlier heads:
                    # part 0 (issued during head 14) heads 0..13,
                    # part 1 (during head 15) head 14,
                    # part 2 (after otn[15]) pair-7 main + corr 15
                    tq = slice(i * 128, (i + 1) * 128)
                    pso = pso_br[i][:, 0:512]
                    if part == 0:
                        oproj_mains(pso, 0, tq, slice(0, 512),
                                    list(range(7)), True)
                        oproj_corrs(pso, 0, tq, slice(0, 512),
                                    list(range(14)), False)
                    elif part == 1:
                        oproj_corrs(pso, 0, tq, slice(0, 512), [14], False)
                    else:
                        oproj_mains(pso, 0, tq, slice(0, 512), [7], False)
                        oproj_corrs(pso, 0, tq, slice(0, 512), [15], True)

                for h in range(H):
                    if h == 0:
                        # o-proj operand prefetches: tile_wait_until keeps them
                        # from being hoisted over the latency-critical wq/x
                        # stream, and 4-head chunks bound how long any one
                        # transfer can block the (exclusive) DMA engines
                        with tc.tile_wait_until(0.086):
                            nc.sync.dma_start(
                                out=bias_sb,
                                in_=bass.AP(
                                    tensor=bo_d, offset=0, ap=[[0, 128], [1, D]]
                                ),
                            )
                        for ci in range(4):
                            with tc.tile_wait_until(0.048 + 0.004 * ci):
                                nc.sync.dma_start(
                                    out=wo_tiles[0][:, 4 * ci:4 * ci + 4],
                                    in_=wo_d[0][:, 4 * ci:4 * ci + 4],
                                )
                        for ci in range(4):
                            with tc.tile_wait_until(0.080 + 0.003 * ci):
                                nc.sync.dma_start(
                                    out=wo_tiles[1][:, 4 * ci:4 * ci + 4],
                                    in_=wo_d[1][:, 4 * ci:4 * ci + 4],
                                )
                    qt = qt_sb[h]
                    otp = ps_op.tile([128, CHUNK], fp32, tag="ps_o")
                    dn = p_dn.tile([128, NQT, 128], fp32, tag="dn")
                    ess = []

                    def attn_unit(t):
                        pss = ps_sp.tile([128, NSIG, 128], fp32, tag="ps_s")
                        qsl = qt[:, t * 128:(t + 1) * 128]
                        for sig in range(NSIG):
                            s = t + sig
                            nc.tensor.matmul(
                                pss[:, sig, :], kt[:, s * 128:(s + 1) * 128], qsl,
                                start=True, stop=True,
                            )
                        es = p_es.tile([128, NSIG, 128], bf16, tag="es")
                        em = p_es.tile([128, 2, 128], bf16, tag="em")
                        ess.append((es, em))
                        # q and k both carry the 2^15 fp8 psum factor; the exp
                        # scale divides it back out
                        nc.scalar.activation(es, pss, AF.Exp, scale=SCALE / (S2 * S2))
                        # only the boundary tiles need masking (em = masked
                        # sig0/sig4); interior tiles are fully in-window, and
                        # fully-padded ones contribute exp(0)=1 per key,
                        # subtracted as an exact count in the last tree op
                        nc.vector.tensor_mul(em[:, 0, :], es[:, 0, :], msk_sb[:, t, :])
                        nc.vector.tensor_mul(em[:, 1, :], es[:, 4, :], msk_sb[:, NQT, :])
                        # fp16 add-tree: 2-byte packed DVE ops run 2x
                        r2 = p_red.tile([128, 2, 128], fp16, tag="r2")
                        nc.gpsimd.tensor_add(r2, em, es[:, 1:3, :])
                        r1 = p_red.tile([128, 128], fp16, tag="r1")
                        nc.vector.tensor_add(r1, r2[:, 0, :], r2[:, 1, :])
                        red = p_red.tile([128, 128], fp16, tag="red")
                        nc.vector.tensor_add(red, r1, es[:, 3, :])
                        nc.gpsimd.partition_all_reduce(
                            dn[:, t, :], red, channels=128,
                            reduce_op=bass_isa.ReduceOp.add,
                        )


                    last_head = h == H - 1
                    if h == H - 2:
                        # the next-next-head q-projection no longer exists to
                        # cover the exp/mask chain; instead accumulate heads
                        # 0..14 of the first two o-proj units into the (free)
                        # ps_q slots, leaving the groups open until otn[15]
                        for i in range(2):
                            t_ = ps_qp.tile(
                                [128, CHUNK], fp32, tag="ps_q", name=f"ps_br{i}"
                            )
                            pso_br.append(t_)

                    if h + 3 < H:
                        nc.sync.dma_start(out=wq_sb[h + 3], in_=wq_d[h + 3])
                    for t in range(2):
                        attn_unit(t)
                    if h + 2 < H:
                        q_proj(ps_qp, h + 2)
                    elif h == H - 2:
                        oproj_partial(0, 0)
                    else:
                        # unit 1's independent prefix first: the corr-14
                        # matmuls wait on the otn[14] quantize chain and would
                        # block the in-order PE queue
                        oproj_partial(1, 0)
                        oproj_partial(0, 1)
                        oproj_partial(1, 1)
                        for pso_u, n_, t_, h2s in u_tail:
                            oproj_corrs(pso_u, n_,
                                        slice(t_ * 128, (t_ + 1) * 128),
                                        slice(0, 512), [14], False)
                    for t in range(2, NQT):
                        attn_unit(t)

                    def av(t):
                        es_t, em_t = ess[t]
                        for sig in range(NSIG):
                            if sig == 0:
                                src = em_t[:, 0, :]
                            elif sig == NSIG - 1:
                                src = em_t[:, 1, :]
                            else:
                                src = es_t[:, sig, :]
                            nc.tensor.matmul(
                                otp[:, t * 128:(t + 1) * 128],
                                v_sb[t + sig], src,
                                start=(sig == 0), stop=(sig == NSIG - 1),
                            )

                    if h == H - 2:
                        # the (n=1, t=0) unit prefix fills the AV-wait bubbles
                        # of head 14 from the spare AV psum buffer
                        pso_u = ps_op.tile([128, CHUNK], fp32, tag="ps_o",
                                           name="u_n1t0")
                        oproj_mains(pso_u, 1, slice(0, 128), slice(0, 512),
                                    list(range(7)), True)
                        oproj_corrs(pso_u, 1, slice(0, 128), slice(0, 512),
                                    list(range(14)), False)
                        u_tail.append((pso_u, 1, 0, [15]))
                    for t in range(NQT):
                        av(t)
                    if last_head:
                        # (n=1, t=1) unit prefix fills PE while the otn[15]
                        # quantize chain drains
                        pst_u = ps_sp.tile([128, NSIG, 128], fp32, tag="ps_s")
                        pso_u = pst_u.rearrange("p s q -> p (s q)")[:, 0:512]
                        oproj_mains(pso_u, 1, slice(128, 256), slice(0, 512),
                                    list(range(7)), True)
                        oproj_corrs(pso_u, 1, slice(128, 256), slice(0, 512),
                                    list(range(15)), False)
                        u_tail.append((pso_u, 1, 1, [15]))
                    rview = dn.rearrange("p t q -> p (t q)")
                    nc.vector.reciprocal_approx_fast(rview, rview)
                    # otn = 64 * normalized head output (vt pre-carried SO/S2);
                    # hi/lo fp8 extraction rides the Pool engine. The last head
                    # splits the chain in half so the o-proj completions that
                    # wait on otn[15] start ~one op earlier.
                    o16 = p_tmp.tile([128, CHUNK], fp16, tag="o16")
                    j_, s_ = divmod(h, 2)
                    halves = (
                        [slice(0, 256), slice(256, CHUNK)] if last_head
                        else [slice(0, CHUNK)]
                    )
                    for cs in halves:
                        nc.vector.tensor_mul(o16[:, cs], otp[:, cs], rview[:, cs])
                        nc.gpsimd.tensor_copy(otn_p[j_][:, s_, 0, cs], o16[:, cs])
                        nc.gpsimd.tensor_sub(
                            otn_p[j_][:, s_, 1, cs], o16[:, cs],
                            otn_p[j_][:, s_, 0, cs]
                        )
                    if last_head:
                        for i in range(2):
                            oproj_partial(i, 2)
                            ob = p_ob.tile([128, 512], bf16, tag="ob")
                            nc.vector.tensor_add(
                                ob, pso_br[i], bias_sb[:, 0:512]
                            )
                            nc.scalar.dma_start(
                                out=out_d[i * 128:(i + 1) * 128, 0:512], in_=ob
                            )

                def oproj_slice(pso, n, t, hs, store_eng=None):
                    tq = slice(t * 128, (t + 1) * 128)
                    oproj_mains(pso[:, hs], n, tq, hs, list(range(H // 2)), True)
                    oproj_corrs(pso[:, hs], n, tq, hs, list(range(H)), True)
                    ob = p_ob.tile([128, 512], bf16, tag="ob")
                    nc.vector.tensor_add(
                        ob[:, hs], pso[:, hs],
                        bias_sb[:, n * 512 + hs.start:n * 512 + hs.stop],
                    )
                    (store_eng or nc.scalar).dma_start(
                        out=out_d[
                            t * 128:(t + 1) * 128,
                            n * 512 + hs.start:n * 512 + hs.stop,
                        ],
                        in_=ob[:, hs],
                    )

                def oproj_unit(pso, n, t):
                    oproj_slice(pso, n, t, slice(0, 512))

                # ---- o-projection + bias, straight out of the (now idle)
                # score-PSUM banks — no pool transition barrier ----
                nc.sync.dma_start(out=wo_tiles[2], in_=wo_d[2])
                nc.sync.dma_start(out=wo_tiles[3], in_=wo_d[3])
                # finish the units whose prefixes ran during heads 14/15
                for pso_u, n_, t_, h2s in u_tail:
                    tq = slice(t_ * 128, (t_ + 1) * 128)
                    oproj_mains(pso_u, n_, tq, slice(0, 512), [7], False)
                    oproj_corrs(pso_u, n_, tq, slice(0, 512), h2s, True)
                    ob = p_ob.tile([128, 512], bf16, tag="ob")
                    nc.vector.tensor_add(
                        ob, pso_u, bias_sb[:, n_ * 512:(n_ + 1) * 512]
                    )
                    nc.scalar.dma_start(
                        out=out_d[tq, n_ * 512:(n_ + 1) * 512], in_=ob
                    )
                for t in range(2, NQT):
                    pst = ps_sp.tile([128, NSIG, 128], fp32, tag="ps_s")
                    pso = pst.rearrange("p s q -> p (s q)")[:, 0:512]
                    oproj_unit(pso, 0, t)
                for n in range(1, DN):
                    for t in range(NQT):
                        if n == 1 and t < 2:
                            continue
                        pst = ps_sp.tile([128, NSIG, 128], fp32, tag="ps_s")
                        pso = pst.rearrange("p s q -> p (s q)")[:, 0:512]
                        if n == DN - 1 and t == NQT - 1:
                            # final unit: the last slice goes in the OTHER
                            # ps_s slot so its matmuls don't wait for the
                            # first slice's bias-add read (same-tile hazard),
                            # and the trailing store chain is short
                            oproj_slice(pso, n, t, slice(0, 384))
                            pst2 = ps_sp.tile(
                                [128, NSIG, 128], fp32, tag="ps_s", name="pst2"
                            )
                            pso2 = pst2.rearrange("p s q -> p (s q)")[:, 0:512]
                            # store via the idle SP queue so it doesn't wait
                            # behind the 384-slice store on Act
                            oproj_slice(pso2, n, t, slice(384, 512),
                                        store_eng=nc.sync)
                        else:
                            oproj_unit(pso, n, t)

    nc.compile()
    return nc


def _get_program():
    global _PROGRAM
    if _PROGRAM is None:
        _PROGRAM = _build_program()
    return _PROGRAM


def _q8pair(a, s):
    """fp8 e4m3 (hi, lo) pair of a*s; lo is quantized at the same scale."""
    hi = (a * s).astype(F8)
    lo = (a * s - hi.astype(np.float32)).astype(F8)
    return hi, lo


def _make_in_maps(x, Wq, Wk, Wv, Wo, bo):
    # host pre-layouts that mirror the SBUF tiles exactly (partition-major,
    # >=512B contiguous per partition) so every DMA runs at full bus rate.
    # q/k/v weights ship as fp8 (lo, hi) pairs in [p, dt, hl, col] layout.
    def wpair(W, ncol):
        hi, lo = _q8pair(np.asarray(W, np.float32), SW)
        # [hl, dt, p, col] -> [p, dt, hl, col]
        st = np.stack([lo, hi]).reshape(2, NDT, 128, ncol)
        return np.ascontiguousarray(st.transpose(2, 1, 0, 3))

    Wq_b = np.ascontiguousarray(
        wpair(Wq, D).reshape(128, NDT, 2, H, 128).transpose(3, 0, 1, 2, 4)
    )
    Wk_b = wpair(Wk, HD)
    Wv_b = wpair(Wv, HD)
    whi, wlo = _q8pair(np.asarray(Wo, np.float32), SWO)
    # [hl, h, p, n, c] -> [n, p, h, hl, c]
    Wo_b = np.ascontiguousarray(
        np.stack([wlo, whi]).reshape(2, H, 128, DN, 512).transpose(3, 2, 1, 0, 4)
    )
    bo_f = np.ascontiguousarray(
        np.asarray(bo, np.float32).reshape(1, D) * (SO * SWO)
    ).astype(BF16)

    inv_freq = np.exp(
        -np.log(np.float32(ROPE_BASE))
        * (np.arange(0, ROPE_DIMS, 2, dtype=np.float32) / np.float32(ROPE_DIMS))
    ).astype(np.float32)

    in_maps = []
    for c in range(8):
        b, g = divmod(c, 4)
        k_start = 512 * g - 512
        xs = np.zeros((NK, D), np.float32)
        lo = max(0, k_start)
        xs[lo - k_start:] = x[b, lo:k_start + NK]
        xhi, xlo = _q8pair(xs.T, SX)
        # [hl, dt, p, pos] -> [p, dt, hl, pos]
        xT = np.ascontiguousarray(
            np.stack([xhi, xlo]).reshape(2, NDT, 128, NK).transpose(2, 1, 0, 3)
        )

        pos = (k_start + np.arange(NK)).astype(np.float32)
        theta = pos[None, :] * inv_freq[:, None]          # [32, NK]
        cos2 = np.ascontiguousarray(
            np.concatenate([np.cos(theta)] * 2, axis=0)).astype(BF16)
        sin2 = np.ascontiguousarray(
            np.concatenate([-np.sin(theta), np.sin(theta)], axis=0)).astype(BF16)

        # boundary masks: sig0 strict-upper triangle (zeroed where sig0 tiles
        # are left-padding, i.e. the first chunk of each batch), sig4 causal
        # lower triangle. Interior tiles are unmasked; fully-padded interior
        # tiles contribute exp(0)=1 to each denominator, subtracted as an
        # exact per-(core, t) count inside the fp16 add tree.
        r_ = np.arange(128)[:, None]   # keys
        q_ = np.arange(128)[None, :]   # queries
        m = np.zeros((NQT + 1, 128, 128), np.float32)
        m[NQT] = q_ >= r_                      # sig4 causal lower triangle
        for t in range(NQT):
            if g > 0:
                m[t] = q_ < r_                 # sig0 strict-upper triangle
            else:
                # sig0 is padding (es0 == 1): carry -npad(t) so the masked
                # product cancels the padded interior tiles' exp(0)=1 sums
                m[t] = -float(sum(1 for sig in range(1, 4) if t + sig < 4))
        masks = np.ascontiguousarray(m.transpose(1, 0, 2)).astype(BF16)

        in_maps.append({
            "xT8": xT, "Wq": Wq_b, "Wk": Wk_b, "Wv": Wv_b, "Wo": Wo_b,
            "bo": bo_f, "cosT": cos2, "sinT": sin2, "masks": masks,
        })
    return in_maps


def _unshard(results):
    out = np.zeros((B, L, D), np.float32)
    for c in range(8):
        b, g = divmod(c, 4)
        # the o-proj psum carries the otn (64x) and Wo (2048x) fp8 scales
        out[b, CHUNK * g:CHUNK * (g + 1)] = (
            results[c]["out"].astype(np.float32) / (SO * SWO)
        )
    return out


def kernel(x, Wq, Wk, Wv, Wo, bo):
    from concourse.bass_utils import run_bass_kernel_spmd

    nc = _get_program()
    in_maps = _make_in_maps(x, Wq, Wk, Wv, Wo, bo)
    res = run_bass_kernel_spmd(nc, in_maps, core_ids=list(range(8)))
    return _unshard(res.results)

